# revision 5
# baseline (speedup 1.0000x reference)
"""NetVLAD Trainium2 kernel (8-core SPMD, data-parallel over batch).

Per-core pipeline (8 samples each):
  mm1:  s^T[k,hw] = W^T @ x^T          (W stationary f16, x^T moving f16)
  ACT:  e^T = exp(s^T + b)             (bias per-partition k)
  PE :  transpose e^T -> e (natural)   (8x [64,128] transposes)
  DVE:  S = sum_k e ; a = e / S        (softmax over k, one broadcast mult)
  mm2:  v^T[k,d] = a^T @ x  (+ diag(a_sum) @ C^T folded into same PSUM accum)
  norms: intra-norm over d and global L2 norm; the only sqrt runs once,
         batched over all samples, to avoid ACT table swaps.
  out:  v_hat^T [64,512] f32 per sample; host transposes to [512*64] layout
"""

import os
import sys

import numpy as np

for _p in ("/opt/trn_rl_repo", "/root/.axon_site/_ro/trn_rl_repo"):
    if os.path.isdir(_p) and _p not in sys.path:
        sys.path.append(_p)

from contextlib import ExitStack

from concourse import bacc, bass, mybir, tile
from concourse.bass_utils import run_bass_kernel_spmd

N_CORES = 8
NSAMP = 8        # samples per core
HW = 1024        # H*W
D = 512
K = 64
DC = D // 128    # d-chunks of 128
HC = HW // 128   # hw-chunks of 128
F16 = mybir.dt.float16
F32 = mybir.dt.float32
EPS = 1e-12
MULT = mybir.AluOpType.mult
ADD = mybir.AluOpType.add

LAST_EXEC_NS = None
LAST_RESULTS = None

_CACHE = {}


def _build_program():
    nc = bacc.Bacc("TRN2", target_bir_lowering=False, debug=False)

    xT_d = nc.dram_tensor("xT", [NSAMP, D, HW], F16, kind="ExternalInput").ap()
    xn_d = nc.dram_tensor("xn", [NSAMP, HW, D], F16, kind="ExternalInput").ap()
    W_d = nc.dram_tensor("Wt", [DC, 128, K], F16, kind="ExternalInput").ap()
    b_d = nc.dram_tensor("bcol", [K, 1], F32, kind="ExternalInput").ap()
    CT_d = nc.dram_tensor("CT", [K, D], F16, kind="ExternalInput").ap()
    eye_d = nc.dram_tensor("eye64", [K, K], F16, kind="ExternalInput").ap()
    ones128_d = nc.dram_tensor("ones128", [128, 1], F16, kind="ExternalInput").ap()
    ones64_d = nc.dram_tensor("ones64", [K, 1], F32, kind="ExternalInput").ap()
    onesr_d = nc.dram_tensor("onesr", [1, K], F32, kind="ExternalInput").ap()
    one1_d = nc.dram_tensor("one1", [1, 1], F16, kind="ExternalInput").ap()
    out_d = nc.dram_tensor("out", [NSAMP, K, D], F32, kind="ExternalOutput").ap()

    with tile.TileContext(nc) as tc, ExitStack() as ctx:
        const = ctx.enter_context(tc.tile_pool(name="const", bufs=1))
        xT_pool = ctx.enter_context(tc.tile_pool(name="xTp", bufs=3))
        xn_pool = ctx.enter_context(tc.tile_pool(name="xnp", bufs=3))
        eT_pool = ctx.enter_context(tc.tile_pool(name="eTp", bufs=2))
        a_pool = ctx.enter_context(tc.tile_pool(name="ap", bufs=2))
        sm_pool = ctx.enter_context(tc.tile_pool(name="smp", bufs=2))
        scr_pool = ctx.enter_context(tc.tile_pool(name="scrp", bufs=2))
        vraw_pool = ctx.enter_context(tc.tile_pool(name="vrawp", bufs=1))
        vo_pool = ctx.enter_context(tc.tile_pool(name="vop", bufs=4))

        ps_sT = ctx.enter_context(tc.tile_pool(name="ps_sT", bufs=2, space="PSUM"))
        ps_e = ctx.enter_context(tc.tile_pool(name="ps_e", bufs=1, space="PSUM"))
        ps_v = ctx.enter_context(tc.tile_pool(name="ps_v", bufs=2, space="PSUM"))
        ps_t = ctx.enter_context(tc.tile_pool(name="ps_t", bufs=1, space="PSUM"))

        # constants
        W_sb = const.tile([128, DC, K], F16)
        nc.sync.dma_start(W_sb[:], W_d[:].rearrange("dc p k -> p dc k"))
        b_sb = const.tile([K, 1], F32)
        nc.sync.dma_start(b_sb[:], b_d[:])
        CT_sb = const.tile([K, D], F16)
        nc.sync.dma_start(CT_sb[:], CT_d[:])
        eye_sb = const.tile([K, K], F16)
        nc.sync.dma_start(eye_sb[:], eye_d[:])
        ones128_sb = const.tile([128, 1], F16)
        nc.sync.dma_start(ones128_sb[:], ones128_d[:])
        ones64_sb = const.tile([K, 1], F32)
        nc.sync.dma_start(ones64_sb[:], ones64_d[:])
        onesr_sb = const.tile([1, K], F32)
        nc.sync.dma_start(onesr_sb[:], onesr_d[:])
        one1_sb = const.tile([1, 1], F16)
        nc.sync.dma_start(one1_sb[:], one1_d[:])
        # shared across samples: the only-sqrt input, one column per sample
        prod_all = const.tile([K, NSAMP], F32)

        vraws = []
        for n in range(NSAMP):
            # ---- loads (chunked for early compute start) ----
            xT_t = xT_pool.tile([128, DC, HW], F16, name=f"xT_{n}", tag="xT")
            for dc in range(DC):
                nc.sync.dma_start(
                    xT_t[:, dc, :], xT_d[n, dc * 128:(dc + 1) * 128, :]
                )
            xn_t = xn_pool.tile([128, HC, D], F16, name=f"xn_{n}", tag="xn")
            for c in range(HC):
                nc.sync.dma_start(
                    xn_t[:, c, :], xn_d[n, c * 128:(c + 1) * 128, :]
                )

            # ---- mm1: s^T = W^T @ x^T  [64, 1024] (dc-outer) ----
            sT_ps = ps_sT.tile([K, HW], F32, name=f"sT_{n}", tag="sT")
            for dc in range(DC):
                for h in range(2):
                    nc.tensor.matmul(
                        sT_ps[:, h * 512:(h + 1) * 512],
                        lhsT=W_sb[:, dc, :],
                        rhs=xT_t[:, dc, h * 512:(h + 1) * 512],
                        start=(dc == 0),
                        stop=(dc == DC - 1),
                    )

            # ---- exp(s^T + b) -> e^T f16 ----
            eT_sb = eT_pool.tile([K, HW], F16, name=f"eT_{n}", tag="eT")
            nc.scalar.activation(
                eT_sb[:], sT_ps[:],
                mybir.ActivationFunctionType.Exp,
                bias=b_sb[:], scale=1.0,
            )

            # ---- transpose e^T -> e natural [128, 8, 64] (PSUM f16) ----
            e_ps = ps_e.tile([128, HC, K], F16, name=f"e_{n}", tag="e")
            for c in range(HC):
                nc.tensor.transpose(
                    e_ps[:, c, :], eT_sb[:, c * 128:(c + 1) * 128], eye_sb[:]
                )

            # ---- softmax denominators + normalize (single broadcast mult) ----
            S_sb = sm_pool.tile([128, HC], F32, name=f"S_{n}", tag="S")
            nc.vector.reduce_sum(S_sb[:], e_ps[:], axis=mybir.AxisListType.X)
            r_sb = sm_pool.tile([128, HC], F32, name=f"r_{n}", tag="r")
            nc.vector.reciprocal(r_sb[:], S_sb[:])
            a_sb = a_pool.tile([128, HC, K], F16, name=f"a_{n}", tag="a")
            nc.vector.tensor_tensor(
                out=a_sb[:], in0=e_ps[:],
                in1=r_sb[:].unsqueeze(-1).broadcast_to((128, HC, K)),
                op=MULT,
            )

            # ---- mm2: v^T = a^T @ x; a_sum via ones-stationary row matmul ----
            v_ps = ps_v.tile([K, D], F32, name=f"v_{n}", tag="v")
            tiny_ps = ps_t.tile([K, 72], F32, name=f"tiny_{n}", tag="tiny")
            for c in range(HC):
                nc.tensor.matmul(
                    v_ps[:],
                    lhsT=a_sb[:, c, :],
                    rhs=xn_t[:, c, :],
                    start=(c == 0),
                    stop=False,
                    skip_group_check=True,
                )
                # a_sum row: [1,64] += ones128^T @ a_chunk (1-col stationary)
                nc.tensor.matmul(
                    tiny_ps[0:1, 0:K],
                    lhsT=ones128_sb[:],
                    rhs=a_sb[:, c, :],
                    start=(c == 0),
                    stop=(c == HC - 1),
                    skip_group_check=True,
                )
            # a_sum row -> column via tiny PE transpose, then diag = eye * a_sum
            arow_sb = sm_pool.tile([1, K], F16, name=f"arow_{n}", tag="arow")
            nc.vector.tensor_copy(arow_sb[:], tiny_ps[0:1, 0:K])
            nc.tensor.matmul(
                tiny_ps[:, 64:65], lhsT=arow_sb[:], rhs=one1_sb[:],
                start=True, stop=True, skip_group_check=True,
            )
            diag_sb = sm_pool.tile([K, K], F16, name=f"diag_{n}", tag="diag")
            nc.vector.tensor_scalar(
                diag_sb[:], eye_sb[:], tiny_ps[:, 64:65], None, op0=MULT,
            )
            nc.tensor.matmul(
                v_ps[:], lhsT=diag_sb[:], rhs=CT_sb[:],
                start=False, stop=True, skip_group_check=True,
            )

            # ---- intra-norm pieces (sqrt deferred to batched tail) ----
            sq_sb = scr_pool.tile([K, D], F16, name=f"sq_{n}", tag="sq")
            ssq_sb = sm_pool.tile([K, 1], F32, name=f"ssq_{n}", tag="ssq")
            nc.scalar.activation(
                sq_sb[:], v_ps[:],
                mybir.ActivationFunctionType.Square,
                accum_out=ssq_sb[:],
            )
            # evacuate raw v^T (kept until tail)
            vraw_sb = vraw_pool.tile([K, D], F32, name=f"vraw_{n}", tag=f"vraw{n}")
            nc.scalar.copy(vraw_sb[:], v_ps[:])
            vraws.append(vraw_sb)

            # t = ssq/(ssq+eps); tot = sum_k t; prod = (ssq+eps)*(tot+eps)
            s1_sb = sm_pool.tile([K, 1], F32, name=f"s1_{n}", tag="s1")
            nc.vector.tensor_scalar(s1_sb[:], ssq_sb[:], EPS, None, op0=ADD)
            rec_sb = sm_pool.tile([K, 1], F32, name=f"rec_{n}", tag="rec")
            nc.vector.reciprocal(rec_sb[:], s1_sb[:])
            t_sb = sm_pool.tile([K, 1], F32, name=f"t_{n}", tag="t")
            nc.vector.tensor_mul(t_sb[:], ssq_sb[:], rec_sb[:])
            nc.tensor.matmul(
                tiny_ps[0:1, 66:67], lhsT=t_sb[:], rhs=ones64_sb[:],
                start=True, stop=True, skip_group_check=True,
            )
            tote_sb = sm_pool.tile([1, 1], F32, name=f"tote_{n}", tag="tote")
            nc.vector.tensor_scalar(tote_sb[:], tiny_ps[0:1, 66:67], EPS, None,
                                    op0=ADD)
            # broadcast tot+eps to all 64 partitions: onesr^T @ tote
            nc.tensor.matmul(
                tiny_ps[:, 68:69], lhsT=onesr_sb[:], rhs=tote_sb[:],
                start=True, stop=True, skip_group_check=True,
            )
            nc.vector.tensor_tensor(
                out=prod_all[:, n:n + 1], in0=s1_sb[:], in1=tiny_ps[:, 68:69],
                op=MULT,
            )

        # ---- tail: the only sqrt (one ACT table swap), then scale + store ----
        sqall_sb = const.tile([K, NSAMP], F32)
        nc.scalar.activation(
            sqall_sb[:], prod_all[:], mybir.ActivationFunctionType.Sqrt,
        )
        alpha_sb = const.tile([K, NSAMP], F32)
        nc.vector.reciprocal(alpha_sb[:], sqall_sb[:])
        for n in range(NSAMP):
            vo_sb = vo_pool.tile([K, D], F32, name=f"vo_{n}", tag="vo")
            nc.scalar.activation(
                vo_sb[:], vraws[n][:],
                mybir.ActivationFunctionType.Copy,
                scale=alpha_sb[:, n:n + 1],
            )
            nc.sync.dma_start(out_d[n], vo_sb[:])

    nc.compile()
    return nc


def _get_program():
    if "nc" not in _CACHE:
        _CACHE["nc"] = _build_program()
    return _CACHE["nc"]


def kernel(x, W_assign, b_assign, C):
    global LAST_EXEC_NS, LAST_RESULTS

    x = np.asarray(x, dtype=np.float32).reshape(64, HW, D)
    W_assign = np.asarray(W_assign, dtype=np.float32)
    b_assign = np.asarray(b_assign, dtype=np.float32)
    C = np.asarray(C, dtype=np.float32)

    W16 = W_assign.astype(np.float16).reshape(DC, 128, K)
    bcol = b_assign.reshape(K, 1)
    CT16 = np.ascontiguousarray(C.T).astype(np.float16)
    eye16 = np.eye(K, dtype=np.float16)
    ones128 = np.ones((128, 1), dtype=np.float16)
    ones64 = np.ones((K, 1), dtype=np.float32)
    onesr = np.ones((1, K), dtype=np.float32)
    one1 = np.ones((1, 1), dtype=np.float16)

    in_maps = []
    for c in range(N_CORES):
        xs = x[c * NSAMP:(c + 1) * NSAMP]
        xn16 = xs.astype(np.float16)
        xT16 = np.ascontiguousarray(xs.transpose(0, 2, 1)).astype(np.float16)
        in_maps.append({
            "xT": xT16, "xn": xn16, "Wt": W16, "bcol": bcol, "CT": CT16,
            "eye64": eye16, "ones128": ones128, "ones64": ones64,
            "onesr": onesr, "one1": one1,
        })

    nc = _get_program()
    trace = bool(int(os.environ.get("KERNEL_TRACE", "0")))
    res = run_bass_kernel_spmd(
        nc, in_maps, core_ids=list(range(N_CORES)), trace=trace,
    )
    LAST_RESULTS = res
    LAST_EXEC_NS = res.exec_time_ns

    out = np.empty((64, D * K), dtype=np.float32)
    for c in range(N_CORES):
        vT = res.results[c]["out"]                    # [NSAMP, K, D]
        out[c * NSAMP:(c + 1) * NSAMP] = (
            vT.transpose(0, 2, 1).reshape(NSAMP, D * K)
        )
    return out


# revision 6
# speedup vs baseline: 1.5287x; 1.5287x over previous
"""NetVLAD Trainium2 kernel (8-core SPMD, data-parallel over batch).

Per-core pipeline (8 samples each):
  mm1:  s^T[k,hw] = W^T @ x^T          (W stationary f16, x^T moving f16)
  ACT:  e^T = exp(s^T + b)             (bias per-partition k)
  PE :  transpose e^T -> e (natural)   (8x [64,128] transposes)
  DVE:  S = sum_k e ; a = e / S        (softmax over k, one broadcast mult)
  mm2:  v^T[k,d] = a^T @ x  (+ diag(a_sum) @ C^T folded into same PSUM accum)
  norms: intra-norm over d and global L2 norm; the only sqrt runs once,
         batched over all samples, to avoid ACT table swaps.
  out:  v_hat^T [64,512] f32 per sample; host transposes to [512*64] layout
"""

import os
import sys

import numpy as np

for _p in ("/opt/trn_rl_repo", "/root/.axon_site/_ro/trn_rl_repo"):
    if os.path.isdir(_p) and _p not in sys.path:
        sys.path.append(_p)

from contextlib import ExitStack

from concourse import bacc, bass, mybir, tile
from concourse.bass_utils import run_bass_kernel_spmd

N_CORES = 8
NSAMP = 8        # samples per core
HW = 1024        # H*W
D = 512
K = 64
DC = D // 128    # d-chunks of 128
HC = HW // 128   # hw-chunks of 128
F16 = mybir.dt.float16
F32 = mybir.dt.float32
EPS = 1e-12
MULT = mybir.AluOpType.mult
ADD = mybir.AluOpType.add

LAST_EXEC_NS = None
LAST_RESULTS = None

_CACHE = {}


def _build_program():
    nc = bacc.Bacc("TRN2", target_bir_lowering=False, debug=False)

    xT_d = nc.dram_tensor("xT", [NSAMP, D, HW], F16, kind="ExternalInput").ap()
    xn_d = nc.dram_tensor("xn", [NSAMP, HW, D], F16, kind="ExternalInput").ap()
    W_d = nc.dram_tensor("Wt", [DC, 128, K], F16, kind="ExternalInput").ap()
    b_d = nc.dram_tensor("bcol", [K, 1], F32, kind="ExternalInput").ap()
    CT_d = nc.dram_tensor("CT", [K, D], F16, kind="ExternalInput").ap()
    eye_d = nc.dram_tensor("eye64", [K, K], F16, kind="ExternalInput").ap()
    ones128_d = nc.dram_tensor("ones128", [128, 1], F16, kind="ExternalInput").ap()
    ones64_d = nc.dram_tensor("ones64", [K, 1], F32, kind="ExternalInput").ap()
    onesr_d = nc.dram_tensor("onesr", [1, K], F32, kind="ExternalInput").ap()
    one1_d = nc.dram_tensor("one1", [1, 1], F16, kind="ExternalInput").ap()
    out_d = nc.dram_tensor("out", [NSAMP, K, D], F32, kind="ExternalOutput").ap()

    with tile.TileContext(nc) as tc, ExitStack() as ctx:
        const = ctx.enter_context(tc.tile_pool(name="const", bufs=1))
        xT_pool = ctx.enter_context(tc.tile_pool(name="xTp", bufs=3))
        xn_pool = ctx.enter_context(tc.tile_pool(name="xnp", bufs=3))
        eT_pool = ctx.enter_context(tc.tile_pool(name="eTp", bufs=2))
        a_pool = ctx.enter_context(tc.tile_pool(name="ap", bufs=2))
        sm_pool = ctx.enter_context(tc.tile_pool(name="smp", bufs=2))
        scr_pool = ctx.enter_context(tc.tile_pool(name="scrp", bufs=2))
        vraw_pool = ctx.enter_context(tc.tile_pool(name="vrawp", bufs=1))
        vo_pool = ctx.enter_context(tc.tile_pool(name="vop", bufs=4))

        ps_sT = ctx.enter_context(tc.tile_pool(name="ps_sT", bufs=2, space="PSUM"))
        ps_e = ctx.enter_context(tc.tile_pool(name="ps_e", bufs=1, space="PSUM"))
        ps_v = ctx.enter_context(tc.tile_pool(name="ps_v", bufs=2, space="PSUM"))
        ps_t = ctx.enter_context(tc.tile_pool(name="ps_t", bufs=1, space="PSUM"))

        # constants
        W_sb = const.tile([128, DC, K], F16)
        nc.sync.dma_start(W_sb[:], W_d[:].rearrange("dc p k -> p dc k"))
        b_sb = const.tile([K, 1], F32)
        nc.sync.dma_start(b_sb[:], b_d[:])
        CT_sb = const.tile([K, D], F16)
        nc.sync.dma_start(CT_sb[:], CT_d[:])
        eye_sb = const.tile([K, K], F16)
        nc.sync.dma_start(eye_sb[:], eye_d[:])
        ones128_sb = const.tile([128, 1], F16)
        nc.sync.dma_start(ones128_sb[:], ones128_d[:])
        ones64_sb = const.tile([K, 1], F32)
        nc.sync.dma_start(ones64_sb[:], ones64_d[:])
        onesr_sb = const.tile([1, K], F32)
        nc.sync.dma_start(onesr_sb[:], onesr_d[:])
        one1_sb = const.tile([1, 1], F16)
        nc.sync.dma_start(one1_sb[:], one1_d[:])
        # shared across samples: the only-sqrt input, one column per sample
        prod_all = const.tile([K, NSAMP], F32)

        vraws = []
        for n in range(NSAMP):
            # ---- loads (chunked for early compute start) ----
            xT_t = xT_pool.tile([128, DC, HW], F16, name=f"xT_{n}", tag="xT")
            nc.sync.dma_start(
                xT_t[:], xT_d[n].rearrange("(dc p) f -> p dc f", dc=DC)
            )
            xn_t = xn_pool.tile([128, HC, D], F16, name=f"xn_{n}", tag="xn")
            nc.sync.dma_start(
                xn_t[:], xn_d[n].rearrange("(c p) d -> p c d", c=HC)
            )

            # ---- mm1: s^T = W^T @ x^T  [64, 1024] (dc-outer) ----
            sT_ps = ps_sT.tile([K, HW], F32, name=f"sT_{n}", tag="sT")
            for h in range(2):
                for dc in range(DC):
                    nc.tensor.matmul(
                        sT_ps[:, h * 512:(h + 1) * 512],
                        lhsT=W_sb[:, dc, :],
                        rhs=xT_t[:, dc, h * 512:(h + 1) * 512],
                        start=(dc == 0),
                        stop=(dc == DC - 1),
                    )

            # ---- exp(s^T + b) -> e^T f16 ----
            eT_sb = eT_pool.tile([K, HW], F16, name=f"eT_{n}", tag="eT")
            nc.scalar.activation(
                eT_sb[:], sT_ps[:],
                mybir.ActivationFunctionType.Exp,
                bias=b_sb[:], scale=1.0,
            )

            # ---- transpose e^T -> e natural [128, 8, 64] (PSUM f16) ----
            e_ps = ps_e.tile([128, HC, K], F16, name=f"e_{n}", tag="e")
            for c in range(HC):
                nc.tensor.transpose(
                    e_ps[:, c, :], eT_sb[:, c * 128:(c + 1) * 128], eye_sb[:]
                )

            # ---- softmax denominators + normalize (single broadcast mult) ----
            S_sb = sm_pool.tile([128, HC], F32, name=f"S_{n}", tag="S")
            nc.vector.reduce_sum(S_sb[:], e_ps[:], axis=mybir.AxisListType.X)
            r_sb = sm_pool.tile([128, HC], F32, name=f"r_{n}", tag="r")
            nc.vector.reciprocal(r_sb[:], S_sb[:])
            a_sb = a_pool.tile([128, HC, K], F16, name=f"a_{n}", tag="a")
            nc.vector.tensor_tensor(
                out=a_sb[:], in0=e_ps[:],
                in1=r_sb[:].unsqueeze(-1).broadcast_to((128, HC, K)),
                op=MULT,
            )

            # ---- mm2: v^T = a^T @ x; a_sum via ones-stationary row matmul ----
            v_ps = ps_v.tile([K, D], F32, name=f"v_{n}", tag="v")
            tiny_ps = ps_t.tile([K, 72], F32, name=f"tiny_{n}", tag="tiny")
            for c in range(HC):
                nc.tensor.matmul(
                    v_ps[:],
                    lhsT=a_sb[:, c, :],
                    rhs=xn_t[:, c, :],
                    start=(c == 0),
                    stop=False,
                    skip_group_check=True,
                )
            for c in range(HC):
                # a_sum row: [1,64] += ones128^T @ a_chunk (1-col stationary)
                nc.tensor.matmul(
                    tiny_ps[0:1, 0:K],
                    lhsT=ones128_sb[:],
                    rhs=a_sb[:, c, :],
                    start=(c == 0),
                    stop=(c == HC - 1),
                    skip_group_check=True,
                )
            # a_sum row -> column via tiny PE transpose, then diag = eye * a_sum
            arow_sb = sm_pool.tile([1, K], F16, name=f"arow_{n}", tag="arow")
            nc.vector.tensor_copy(arow_sb[:], tiny_ps[0:1, 0:K])
            nc.tensor.matmul(
                tiny_ps[:, 64:65], lhsT=arow_sb[:], rhs=one1_sb[:],
                start=True, stop=True, skip_group_check=True,
            )
            diag_sb = sm_pool.tile([K, K], F16, name=f"diag_{n}", tag="diag")
            nc.vector.tensor_scalar(
                diag_sb[:], eye_sb[:], tiny_ps[:, 64:65], None, op0=MULT,
            )
            nc.tensor.matmul(
                v_ps[:], lhsT=diag_sb[:], rhs=CT_sb[:],
                start=False, stop=True, skip_group_check=True,
            )

            # ---- intra-norm pieces (sqrt deferred to batched tail) ----
            sq_sb = scr_pool.tile([K, D], F16, name=f"sq_{n}", tag="sq")
            ssq_sb = sm_pool.tile([K, 1], F32, name=f"ssq_{n}", tag="ssq")
            nc.scalar.activation(
                sq_sb[:], v_ps[:],
                mybir.ActivationFunctionType.Square,
                accum_out=ssq_sb[:],
            )
            # evacuate raw v^T (kept until tail)
            vraw_sb = vraw_pool.tile([K, D], F32, name=f"vraw_{n}", tag=f"vraw{n}")
            nc.scalar.copy(vraw_sb[:], v_ps[:])
            vraws.append(vraw_sb)

            # t = ssq/(ssq+eps); tot = sum_k t; prod = (ssq+eps)*(tot+eps)
            s1_sb = sm_pool.tile([K, 1], F32, name=f"s1_{n}", tag="s1")
            nc.vector.tensor_scalar(s1_sb[:], ssq_sb[:], EPS, None, op0=ADD)
            rec_sb = sm_pool.tile([K, 1], F32, name=f"rec_{n}", tag="rec")
            nc.vector.reciprocal(rec_sb[:], s1_sb[:])
            t_sb = sm_pool.tile([K, 1], F32, name=f"t_{n}", tag="t")
            nc.vector.tensor_mul(t_sb[:], ssq_sb[:], rec_sb[:])
            nc.tensor.matmul(
                tiny_ps[0:1, 66:67], lhsT=t_sb[:], rhs=ones64_sb[:],
                start=True, stop=True, skip_group_check=True,
            )
            tote_sb = sm_pool.tile([1, 1], F32, name=f"tote_{n}", tag="tote")
            nc.vector.tensor_scalar(tote_sb[:], tiny_ps[0:1, 66:67], EPS, None,
                                    op0=ADD)
            # broadcast tot+eps to all 64 partitions: onesr^T @ tote
            nc.tensor.matmul(
                tiny_ps[:, 68:69], lhsT=onesr_sb[:], rhs=tote_sb[:],
                start=True, stop=True, skip_group_check=True,
            )
            nc.vector.tensor_tensor(
                out=prod_all[:, n:n + 1], in0=s1_sb[:], in1=tiny_ps[:, 68:69],
                op=MULT,
            )

        # ---- tail: the only sqrt (one ACT table swap), then scale + store ----
        sqall_sb = const.tile([K, NSAMP], F32)
        nc.scalar.activation(
            sqall_sb[:], prod_all[:], mybir.ActivationFunctionType.Sqrt,
        )
        alpha_sb = const.tile([K, NSAMP], F32)
        nc.vector.reciprocal(alpha_sb[:], sqall_sb[:])
        for n in range(NSAMP):
            vo_sb = vo_pool.tile([K, D], F32, name=f"vo_{n}", tag="vo")
            nc.scalar.activation(
                vo_sb[:], vraws[n][:],
                mybir.ActivationFunctionType.Copy,
                scale=alpha_sb[:, n:n + 1],
            )
            nc.gpsimd.dma_start(out_d[n], vo_sb[:])

    nc.compile()
    return nc


def _get_program():
    if "nc" not in _CACHE:
        _CACHE["nc"] = _build_program()
    return _CACHE["nc"]


def kernel(x, W_assign, b_assign, C):
    global LAST_EXEC_NS, LAST_RESULTS

    x = np.asarray(x, dtype=np.float32).reshape(64, HW, D)
    W_assign = np.asarray(W_assign, dtype=np.float32)
    b_assign = np.asarray(b_assign, dtype=np.float32)
    C = np.asarray(C, dtype=np.float32)

    W16 = W_assign.astype(np.float16).reshape(DC, 128, K)
    bcol = b_assign.reshape(K, 1)
    CT16 = np.ascontiguousarray(C.T).astype(np.float16)
    eye16 = np.eye(K, dtype=np.float16)
    ones128 = np.ones((128, 1), dtype=np.float16)
    ones64 = np.ones((K, 1), dtype=np.float32)
    onesr = np.ones((1, K), dtype=np.float32)
    one1 = np.ones((1, 1), dtype=np.float16)

    in_maps = []
    for c in range(N_CORES):
        xs = x[c * NSAMP:(c + 1) * NSAMP]
        xn16 = xs.astype(np.float16)
        xT16 = np.ascontiguousarray(xs.transpose(0, 2, 1)).astype(np.float16)
        in_maps.append({
            "xT": xT16, "xn": xn16, "Wt": W16, "bcol": bcol, "CT": CT16,
            "eye64": eye16, "ones128": ones128, "ones64": ones64,
            "onesr": onesr, "one1": one1,
        })

    nc = _get_program()
    trace = bool(int(os.environ.get("KERNEL_TRACE", "0")))
    res = run_bass_kernel_spmd(
        nc, in_maps, core_ids=list(range(N_CORES)), trace=trace,
    )
    LAST_RESULTS = res
    LAST_EXEC_NS = res.exec_time_ns

    out = np.empty((64, D * K), dtype=np.float32)
    for c in range(N_CORES):
        vT = res.results[c]["out"]                    # [NSAMP, K, D]
        out[c * NSAMP:(c + 1) * NSAMP] = (
            vT.transpose(0, 2, 1).reshape(NSAMP, D * K)
        )
    return out


# revision 7
# speedup vs baseline: 1.6437x; 1.0752x over previous
"""NetVLAD Trainium2 kernel (8-core SPMD, data-parallel over batch).

Per-core pipeline (8 samples each):
  mm1:  s^T[k,hw] = W^T @ x^T          (W stationary f16, x^T moving f16)
  ACT:  e^T = exp(s^T + b)             (bias per-partition k)
  PE :  transpose e^T -> e (natural)   (8x [64,128] transposes)
  DVE:  S = sum_k e ; a = e / S        (softmax over k, one broadcast mult)
  mm2:  v^T[k,d] = a^T @ x  (+ diag(a_sum) @ C^T folded into same PSUM accum)
  norms: intra-norm over d and global L2 norm; the only sqrt runs once,
         batched over all samples, to avoid ACT table swaps.
  out:  v_hat^T [64,512] f32 per sample; host transposes to [512*64] layout
"""

import os
import sys

import numpy as np

for _p in ("/opt/trn_rl_repo", "/root/.axon_site/_ro/trn_rl_repo"):
    if os.path.isdir(_p) and _p not in sys.path:
        sys.path.append(_p)

from contextlib import ExitStack

from concourse import bacc, bass, mybir, tile
from concourse.bass_utils import run_bass_kernel_spmd

N_CORES = 8
NSAMP = 8        # samples per core
HW = 1024        # H*W
D = 512
K = 64
DC = D // 128    # d-chunks of 128
HC = HW // 128   # hw-chunks of 128
F16 = mybir.dt.float16
F32 = mybir.dt.float32
EPS = 1e-12
MULT = mybir.AluOpType.mult
ADD = mybir.AluOpType.add

LAST_EXEC_NS = None
LAST_RESULTS = None

_CACHE = {}


def _build_program():
    nc = bacc.Bacc("TRN2", target_bir_lowering=False, debug=False)

    xT_d = nc.dram_tensor("xT", [NSAMP, 128, DC * HW], F16, kind="ExternalInput").ap()
    xn_d = nc.dram_tensor("xn", [NSAMP, 128, HC * D], F16, kind="ExternalInput").ap()
    W_d = nc.dram_tensor("Wt", [128, DC * K], F16, kind="ExternalInput").ap()
    b_d = nc.dram_tensor("bcol", [K, 1], F32, kind="ExternalInput").ap()
    CT_d = nc.dram_tensor("CT", [K, D], F16, kind="ExternalInput").ap()
    eye_d = nc.dram_tensor("eye64", [K, K], F16, kind="ExternalInput").ap()
    ones128_d = nc.dram_tensor("ones128", [128, 1], F16, kind="ExternalInput").ap()
    ones64_d = nc.dram_tensor("ones64", [K, 1], F32, kind="ExternalInput").ap()
    onesr_d = nc.dram_tensor("onesr", [1, K], F32, kind="ExternalInput").ap()
    one1_d = nc.dram_tensor("one1", [1, 1], F16, kind="ExternalInput").ap()
    out_d = nc.dram_tensor("out", [NSAMP, K, D], F32, kind="ExternalOutput").ap()

    with tile.TileContext(nc) as tc, ExitStack() as ctx:
        const = ctx.enter_context(tc.tile_pool(name="const", bufs=1))
        xT_pool = ctx.enter_context(tc.tile_pool(name="xTp", bufs=3))
        xn_pool = ctx.enter_context(tc.tile_pool(name="xnp", bufs=3))
        eT_pool = ctx.enter_context(tc.tile_pool(name="eTp", bufs=2))
        a_pool = ctx.enter_context(tc.tile_pool(name="ap", bufs=2))
        sm_pool = ctx.enter_context(tc.tile_pool(name="smp", bufs=2))
        scr_pool = ctx.enter_context(tc.tile_pool(name="scrp", bufs=2))
        vraw_pool = ctx.enter_context(tc.tile_pool(name="vrawp", bufs=1))
        vo_pool = ctx.enter_context(tc.tile_pool(name="vop", bufs=4))

        ps_sT = ctx.enter_context(tc.tile_pool(name="ps_sT", bufs=2, space="PSUM"))
        ps_e = ctx.enter_context(tc.tile_pool(name="ps_e", bufs=1, space="PSUM"))
        ps_v = ctx.enter_context(tc.tile_pool(name="ps_v", bufs=2, space="PSUM"))
        ps_t = ctx.enter_context(tc.tile_pool(name="ps_t", bufs=1, space="PSUM"))

        # constants
        W_sb = const.tile([128, DC, K], F16)
        nc.sync.dma_start(W_sb[:].rearrange("p dc k -> p (dc k)"), W_d[:])
        b_sb = const.tile([K, 1], F32)
        nc.sync.dma_start(b_sb[:], b_d[:])
        CT_sb = const.tile([K, D], F16)
        nc.sync.dma_start(CT_sb[:], CT_d[:])
        eye_sb = const.tile([K, K], F16)
        nc.sync.dma_start(eye_sb[:], eye_d[:])
        ones128_sb = const.tile([128, 1], F16)
        nc.sync.dma_start(ones128_sb[:], ones128_d[:])
        ones64_sb = const.tile([K, 1], F32)
        nc.sync.dma_start(ones64_sb[:], ones64_d[:])
        onesr_sb = const.tile([1, K], F32)
        nc.sync.dma_start(onesr_sb[:], onesr_d[:])
        one1_sb = const.tile([1, 1], F16)
        nc.sync.dma_start(one1_sb[:], one1_d[:])
        # shared across samples: the only-sqrt input, one column per sample
        prod_all = const.tile([K, NSAMP], F32)

        vraws = []
        for n in range(NSAMP):
            # ---- loads (chunked for early compute start) ----
            xT_t = xT_pool.tile([128, DC, HW], F16, name=f"xT_{n}", tag="xT")
            nc.sync.dma_start(xT_t[:].rearrange("p dc f -> p (dc f)"), xT_d[n])
            xn_t = xn_pool.tile([128, HC, D], F16, name=f"xn_{n}", tag="xn")
            nc.sync.dma_start(xn_t[:].rearrange("p c d -> p (c d)"), xn_d[n])

            # ---- mm1: s^T = W^T @ x^T  [64, 1024] (dc-outer) ----
            sT_ps = ps_sT.tile([K, HW], F32, name=f"sT_{n}", tag="sT")
            for h in range(2):
                for dc in range(DC):
                    nc.tensor.matmul(
                        sT_ps[:, h * 512:(h + 1) * 512],
                        lhsT=W_sb[:, dc, :],
                        rhs=xT_t[:, dc, h * 512:(h + 1) * 512],
                        start=(dc == 0),
                        stop=(dc == DC - 1),
                    )

            # ---- exp(s^T + b) -> e^T f16 ----
            eT_sb = eT_pool.tile([K, HW], F16, name=f"eT_{n}", tag="eT")
            nc.scalar.activation(
                eT_sb[:], sT_ps[:],
                mybir.ActivationFunctionType.Exp,
                bias=b_sb[:], scale=1.0,
            )

            # ---- transpose e^T -> e natural [128, 8, 64] (PSUM f16) ----
            e_ps = ps_e.tile([128, HC, K], F16, name=f"e_{n}", tag="e")
            for c in range(HC):
                nc.tensor.transpose(
                    e_ps[:, c, :], eT_sb[:, c * 128:(c + 1) * 128], eye_sb[:]
                )

            # ---- softmax denominators + normalize (single broadcast mult) ----
            S_sb = sm_pool.tile([128, HC], F32, name=f"S_{n}", tag="S")
            nc.vector.reduce_sum(S_sb[:], e_ps[:], axis=mybir.AxisListType.X)
            r_sb = sm_pool.tile([128, HC], F32, name=f"r_{n}", tag="r")
            nc.vector.reciprocal(r_sb[:], S_sb[:])
            a_sb = a_pool.tile([128, HC, K], F16, name=f"a_{n}", tag="a")
            nc.vector.tensor_tensor(
                out=a_sb[:], in0=e_ps[:],
                in1=r_sb[:].unsqueeze(-1).broadcast_to((128, HC, K)),
                op=MULT,
            )

            # ---- mm2: v^T = a^T @ x; a_sum via ones-stationary row matmul ----
            v_ps = ps_v.tile([K, D], F32, name=f"v_{n}", tag="v")
            tiny_ps = ps_t.tile([K, 72], F32, name=f"tiny_{n}", tag="tiny")
            for c in range(HC):
                nc.tensor.matmul(
                    v_ps[:],
                    lhsT=a_sb[:, c, :],
                    rhs=xn_t[:, c, :],
                    start=(c == 0),
                    stop=False,
                    skip_group_check=True,
                )
            for c in range(HC):
                # a_sum row: [1,64] += ones128^T @ a_chunk (1-col stationary)
                nc.tensor.matmul(
                    tiny_ps[0:1, 0:K],
                    lhsT=ones128_sb[:],
                    rhs=a_sb[:, c, :],
                    start=(c == 0),
                    stop=(c == HC - 1),
                    skip_group_check=True,
                )
            # a_sum row -> column via tiny PE transpose, then diag = eye * a_sum
            arow_sb = sm_pool.tile([1, K], F16, name=f"arow_{n}", tag="arow")
            nc.vector.tensor_copy(arow_sb[:], tiny_ps[0:1, 0:K])
            nc.tensor.matmul(
                tiny_ps[:, 64:65], lhsT=arow_sb[:], rhs=one1_sb[:],
                start=True, stop=True, skip_group_check=True,
            )
            diag_sb = sm_pool.tile([K, K], F16, name=f"diag_{n}", tag="diag")
            nc.vector.tensor_scalar(
                diag_sb[:], eye_sb[:], tiny_ps[:, 64:65], None, op0=MULT,
            )
            nc.tensor.matmul(
                v_ps[:], lhsT=diag_sb[:], rhs=CT_sb[:],
                start=False, stop=True, skip_group_check=True,
            )

            # ---- intra-norm pieces (sqrt deferred to batched tail) ----
            sq_sb = scr_pool.tile([K, D], F16, name=f"sq_{n}", tag="sq")
            ssq_sb = sm_pool.tile([K, 1], F32, name=f"ssq_{n}", tag="ssq")
            nc.scalar.activation(
                sq_sb[:], v_ps[:],
                mybir.ActivationFunctionType.Square,
                accum_out=ssq_sb[:],
            )
            # evacuate raw v^T (kept until tail)
            vraw_sb = vraw_pool.tile([K, D], F32, name=f"vraw_{n}", tag=f"vraw{n}")
            nc.scalar.copy(vraw_sb[:], v_ps[:])
            vraws.append(vraw_sb)

            # t = ssq/(ssq+eps); tot = sum_k t; prod = (ssq+eps)*(tot+eps)
            s1_sb = sm_pool.tile([K, 1], F32, name=f"s1_{n}", tag="s1")
            nc.vector.tensor_scalar(s1_sb[:], ssq_sb[:], EPS, None, op0=ADD)
            rec_sb = sm_pool.tile([K, 1], F32, name=f"rec_{n}", tag="rec")
            nc.vector.reciprocal(rec_sb[:], s1_sb[:])
            t_sb = sm_pool.tile([K, 1], F32, name=f"t_{n}", tag="t")
            nc.vector.tensor_mul(t_sb[:], ssq_sb[:], rec_sb[:])
            nc.tensor.matmul(
                tiny_ps[0:1, 66:67], lhsT=t_sb[:], rhs=ones64_sb[:],
                start=True, stop=True, skip_group_check=True,
            )
            tote_sb = sm_pool.tile([1, 1], F32, name=f"tote_{n}", tag="tote")
            nc.vector.tensor_scalar(tote_sb[:], tiny_ps[0:1, 66:67], EPS, None,
                                    op0=ADD)
            # broadcast tot+eps to all 64 partitions: onesr^T @ tote
            nc.tensor.matmul(
                tiny_ps[:, 68:69], lhsT=onesr_sb[:], rhs=tote_sb[:],
                start=True, stop=True, skip_group_check=True,
            )
            nc.vector.tensor_tensor(
                out=prod_all[:, n:n + 1], in0=s1_sb[:], in1=tiny_ps[:, 68:69],
                op=MULT,
            )

        # ---- tail: the only sqrt (one ACT table swap), then scale + store ----
        sqall_sb = const.tile([K, NSAMP], F32)
        nc.scalar.activation(
            sqall_sb[:], prod_all[:], mybir.ActivationFunctionType.Sqrt,
        )
        alpha_sb = const.tile([K, NSAMP], F32)
        nc.vector.reciprocal(alpha_sb[:], sqall_sb[:])
        for n in range(NSAMP):
            vo_sb = vo_pool.tile([K, D], F32, name=f"vo_{n}", tag="vo")
            if n % 2 == 0:
                nc.scalar.activation(
                    vo_sb[:], vraws[n][:],
                    mybir.ActivationFunctionType.Copy,
                    scale=alpha_sb[:, n:n + 1],
                )
            else:
                nc.vector.tensor_scalar(
                    vo_sb[:], vraws[n][:], alpha_sb[:, n:n + 1], None, op0=MULT,
                )
            nc.sync.dma_start(out_d[n], vo_sb[:])

    nc.compile()
    return nc


def _get_program():
    if "nc" not in _CACHE:
        _CACHE["nc"] = _build_program()
    return _CACHE["nc"]


def kernel(x, W_assign, b_assign, C):
    global LAST_EXEC_NS, LAST_RESULTS

    x = np.asarray(x, dtype=np.float32).reshape(64, HW, D)
    W_assign = np.asarray(W_assign, dtype=np.float32)
    b_assign = np.asarray(b_assign, dtype=np.float32)
    C = np.asarray(C, dtype=np.float32)

    W16 = np.ascontiguousarray(
        W_assign.astype(np.float16).reshape(DC, 128, K).transpose(1, 0, 2)
    ).reshape(128, DC * K)
    bcol = b_assign.reshape(K, 1)
    CT16 = np.ascontiguousarray(C.T).astype(np.float16)
    eye16 = np.eye(K, dtype=np.float16)
    ones128 = np.ones((128, 1), dtype=np.float16)
    ones64 = np.ones((K, 1), dtype=np.float32)
    onesr = np.ones((1, K), dtype=np.float32)
    one1 = np.ones((1, 1), dtype=np.float16)

    in_maps = []
    for c in range(N_CORES):
        xs = x[c * NSAMP:(c + 1) * NSAMP]
        # xn sbuf image: [n, p, c, d] from [n, (c p), d]
        xn16 = np.ascontiguousarray(
            xs.reshape(NSAMP, HC, 128, D).transpose(0, 2, 1, 3)
        ).reshape(NSAMP, 128, HC * D).astype(np.float16)
        # xT sbuf image: [n, p, dc, hw] from [n, (dc p), hw] of x^T
        xT = xs.transpose(0, 2, 1).reshape(NSAMP, DC, 128, HW)
        xT16 = np.ascontiguousarray(
            xT.transpose(0, 2, 1, 3)
        ).reshape(NSAMP, 128, DC * HW).astype(np.float16)
        in_maps.append({
            "xT": xT16, "xn": xn16, "Wt": W16, "bcol": bcol, "CT": CT16,
            "eye64": eye16, "ones128": ones128, "ones64": ones64,
            "onesr": onesr, "one1": one1,
        })

    nc = _get_program()
    trace = bool(int(os.environ.get("KERNEL_TRACE", "0")))
    res = run_bass_kernel_spmd(
        nc, in_maps, core_ids=list(range(N_CORES)), trace=trace,
    )
    LAST_RESULTS = res
    LAST_EXEC_NS = res.exec_time_ns

    out = np.empty((64, D * K), dtype=np.float32)
    for c in range(N_CORES):
        vT = res.results[c]["out"]                    # [NSAMP, K, D]
        out[c * NSAMP:(c + 1) * NSAMP] = (
            vT.transpose(0, 2, 1).reshape(NSAMP, D * K)
        )
    return out
